# revision 5
# baseline (speedup 1.0000x reference)
"""LSTM decoder (nn_Decoder) on 8 trn2 NeuronCores — remote-DMA exchange.

Tensor-parallel over the 4H gate dimension, with all state in transposed
([channel, batch]) layout so the per-step hidden-state all-gather is a
direct SBUF->SBUF remote_dma_broadcast (no ncfw collective, no DRAM
bounce, no transpose DMAs).

Reference recurrence: x_t = h_t after step 1, so steps >= 2 use
W = w_ih + w_hh (host-side sum); step 1 (x0 = 0) uses w_hh alone.

Per core (rank q owns H-channels [128q, 128q+128) and their 4 gate rows):
  - gates.T tiles f,i,g,o: 32 matmuls, stationary = weight tile [128,128]
    bf16 (FWL), moving = gathered h.T chunk [128, 64]; arrival waits are
    interleaved into the first tile so they hide under the matmul stream.
  - gate biases ride the ACT activations as per-partition bias vectors.
  - c/h updates on [128, 64] tiles (DVE), h emitted as bf16 [128, 64].
  - h.T chunk broadcast to slot q of a double-buffered [128, 512] gather
    buffer on all 8 cores (If-chain on a rank register; per-sender remote
    semaphores). Descriptors are prepped during the previous step's
    flight; the trigger is gated on the DVE c-write (the tanh_c + h-mult
    ~0.65us run concurrently with the ~1.3us Q7-observe + trigger + SDMA
    descriptor fetch, so the hbf write still wins by ~0.6us).
  - output projection out[t] = h_{t+1} @ out_w_q.T rides the gather slots
    as stationary one round behind; out_b via a K=1 ones-matmul. Dummy
    matmuls keep the PE HAM clock warm through the exchange window.
  - the ncfw AllGather at the top is never waited on mid-path: its mere
    presence makes the runtime align core starts with a pre-exec barrier.

Measured: ~1.44 ms (vs 4.78 ms ncfw-collective baseline), rel err 4.5e-3.
"""
import os
import sys

sys.path.insert(0, "/opt/trn_rl_repo")

import numpy as np
import ml_dtypes

BF16 = ml_dtypes.bfloat16

B = 64          # batch
L = 256         # latent dim
H = 1024        # hidden
O = 512         # output dim
S = 256         # seq len
NC = 8          # cores
HL = H // NC    # 128, per-core h slice
OL = O // NC    # 64, per-core out slice
NDUM = 7        # dummy matmuls per round (HAM warmth during flight)


def _build_nc(s_len):
    from concourse import bass, mybir
    from concourse import bacc

    S_ = s_len
    nc = bacc.Bacc("TRN2", debug=False)
    f32 = mybir.dt.float32
    bf16 = mybir.dt.bfloat16
    i32 = mybir.dt.int32
    AF = mybir.ActivationFunctionType
    ALU = mybir.AluOpType

    d_lat = nc.dram_tensor("latT", [128, 2 * B], bf16, kind="ExternalInput")
    d_fcw = nc.dram_tensor("fcwT", [128, 2 * HL], bf16, kind="ExternalInput")
    d_bia = nc.dram_tensor("biasv", [128, 8], f32, kind="ExternalInput")
    d_msc = nc.dram_tensor("msc", [1, 192], bf16, kind="ExternalInput")
    d_rank = nc.dram_tensor("rank", [1, 2], i32, kind="ExternalInput")
    d_whh = nc.dram_tensor("whhT", [128, 32 * 128], bf16, kind="ExternalInput")
    d_wc = nc.dram_tensor("wcT", [128, 32 * 128], bf16, kind="ExternalInput")
    d_ow = nc.dram_tensor("owT", [128, NC * OL], bf16, kind="ExternalInput")
    d_out = nc.dram_tensor("out", [B, S_ * OL], f32, kind="ExternalOutput")

    cc_in = nc.dram_tensor("cc_in", [1, 8], f32)
    cc_out = nc.dram_tensor("cc_out", [8, 8], f32, addr_space="Shared")

    from contextlib import ExitStack
    ctx = ExitStack()
    sem = lambda n: ctx.enter_context(nc.semaphore(n))
    sb = lambda n, sh, dt: ctx.enter_context(nc.sbuf_tensor(n, sh, dt))
    ps = lambda n, sh, dt: ctx.enter_context(nc.psum_tensor(n, sh, dt))

    in_dma = sem("in_dma")    # input loads, +16 each
    bsem = sem("bsem")        # startup barrier
    gsems = [sem(f"gsem{q}") for q in range(NC)]  # +2 per exchange from rank q
    psem = sem("psem")        # desc-gen done, = e+1
    lsem = sem("lsem")        # send complete, = 16(e+1)
    hsem = sem("hsem")        # h_e ready, = e+1
    mmq = sem("mmq")          # PE: h0 -> 1; round r tile t -> 1+4(r-1)+t+1
    actsem = sem("actsem")    # ACT: 5 per round
    dvesem = sem("dvesem")    # DVE c ready, = r
    osem = sem("osem")        # outproj psum done, = r-1
    ocp = sem("ocp")          # out copy done, = r-1
    pesem = sem("pesem")      # PE round done (incl outproj), = r
    odma = sem("odma")        # output flush dmas

    lat_sb = sb("lat_sb", [128, 2 * B], bf16)
    fcw_sb = sb("fcw_sb", [128, 2 * HL], bf16)
    bia_sb = sb("bia_sb", [128, 8], f32)
    msc_sb = sb("msc_sb", [1, 192], bf16)
    rank_sb = sb("rank_sb", [1, 2], i32)
    whh_sb = sb("whh_sb", [128, 32 * 128], bf16)
    wc_sb = sb("wc_sb", [128, 32 * 128], bf16)
    ow_sb = sb("ow_sb", [128, NC * OL], bf16)
    gather = [sb("gather0", [128, NC * B], bf16),
              sb("gather1", [128, NC * B], bf16)]
    hbf = [sb("hbf0", [128, B], bf16), sb("hbf1", [128, B], bf16)]
    f_sb = sb("f_sb", [128, B], f32)
    i_sb = sb("i_sb", [128, B], f32)
    g_sb = sb("g_sb", [128, B], f32)
    o_sb = sb("o_sb", [128, B], f32)
    t1_sb = sb("t1_sb", [128, B], f32)
    t2_sb = sb("t2_sb", [128, B], f32)
    c_sb = sb("c_sb", [128, B], f32)
    tanhc_sb = sb("tanhc_sb", [128, B], f32)
    out_acc = sb("out_acc", [B, S_ * OL], f32)

    ps_t = [ps(f"ps_{n}", [128, B], f32) for n in "figo"]
    ps_h0 = ps("ps_h0", [128, B], f32)
    ps_out = ps("ps_out", [B, OL], f32)
    ps_dum = ps("ps_dum", [128, 512], f32)

    OUT_CHUNK = 32
    n_chunks = S_ // OUT_CHUNK

    with nc.Block() as block:

        @block.sync
        def _(sync):
            for dram, buf in ((d_lat, lat_sb), (d_fcw, fcw_sb),
                              (d_bia, bia_sb), (d_msc, msc_sb),
                              (d_rank, rank_sb), (d_whh, whh_sb),
                              (d_wc, wc_sb), (d_ow, ow_sb)):
                sync.dma_start(buf[:, :], dram[:, :]).then_inc(in_dma, 16)
            # two-part flush: first half mid-loop (one contention event),
            # second half at the end (shorter tail)
            HF = S_ // 2
            sync.wait_ge(ocp, HF)
            sync.dma_start(d_out[:, :HF * OL],
                           out_acc[:, :HF * OL]).then_inc(odma, 16)
            sync.wait_ge(ocp, S_)
            sync.dma_start(d_out[:, HF * OL:],
                           out_acc[:, HF * OL:]).then_inc(odma, 16)
            sync.wait_ge(odma, 32)

        @block.gpsimd
        def _(gp):
            gp.collective_compute(
                "AllGather", mybir.AluOpType.bypass,
                replica_groups=[list(range(NC))],
                ins=[cc_in.ap().opt()], outs=[cc_out.ap().opt()],
            ).then_inc(bsem)
            # no wait on bsem here: the NEFF-level pre-execution barrier
            # (present because this collective exists) already aligns core
            # starts; exchanges ride plain SDMA and need no ncfw
            gp.wait_ge(in_dma, 128)          # all inputs loaded
            r = gp.alloc_register("rankreg")
            gp.reg_load(r, rank_sb[0:1, 0:1])
            rdests = [(0, k) for k in range(NC)]
            for e in range(S_ + 1):
                gb = gather[e % 2]

                def bc(q):
                    gp.remote_dma_broadcast(
                        gb[:, q * B:(q + 1) * B], hbf[e % 2][:, :],
                        remote_sem=gsems[q], local_sem=lsem, rdests=rdests,
                    ).then_inc(psem, 1)

                def chain(lo, hi):
                    if hi - lo == 1:
                        bc(lo)
                        return
                    mid = (lo + hi) // 2
                    with gp.If_lt(r, mid):
                        chain(lo, mid)
                    with gp.Else():
                        chain(mid, hi)

                chain(0, NC)
                gp.wait_ge(psem, e + 1)
                if e >= 2:
                    for q in range(NC):
                        gp.wait_ge(gsems[q], 2 * (e - 1))
                if e >= 2:
                    gp.wait_ge(lsem, 16 * (e - 1))
                if e >= 1:
                    gp.wait_ge(pesem, e)     # peers' gather readers done
                if e == 0:
                    gp.wait_ge(hsem, 1)      # h_0 ready
                else:
                    # early trigger: c done; tanh_c + h-mult (~0.65us) run
                    # while the Q7 observe+trigger+SDMA descriptor fetch
                    # (~1.3us) proceeds — hbf write wins by ~0.6us
                    gp.wait_ge(dvesem, e)
                gp.trigger_dma(count=1)
            gp.wait_ge(bsem, 1)

        @block.tensor
        def _(te):
            mm = te.matmul
            te.wait_ge(in_dma, 128)
            mm(ps_h0[:, :], fcw_sb[:, 0:HL], lat_sb[:, 0:B],
               start=True, stop=False)
            mm(ps_h0[:, :], fcw_sb[:, HL:2 * HL], lat_sb[:, B:2 * B],
               start=False, stop=True).then_inc(mmq)       # mmq = 1

            def dummy(n):
                for _ in range(n):
                    mm(ps_dum[:, :], wc_sb[:, 0:128], wc_sb[:, 128:640],
                       start=True, stop=True)

            def small_dummy():
                return mm(ps_dum[:, 0:B], wc_sb[:, 0:128], wc_sb[:, 128:192],
                          start=True, stop=True)

            for r in range(1, S_ + 1):
                gp_buf = gather[(r - 1) % 2]
                W = whh_sb if r == 1 else wc_sb
                for t in range(4):
                    for j in range(NC):
                        if t == 0:
                            # interleaved arrival waits: wait j hides under MM j-1
                            te.wait_ge(gsems[j], 2 * r)
                        inst = mm(ps_t[t][:, :],
                                  W[:, (t * 8 + j) * 128:(t * 8 + j + 1) * 128],
                                  gp_buf[:, j * B:(j + 1) * B],
                                  start=(j == 0), stop=(j == NC - 1))
                        if j == NC - 1:
                            inst.then_inc(mmq)   # = 1 + 4(r-1) + t + 1
                if r >= 2:
                    te.wait_ge(ocp, r - 2)
                    mm(ps_out[:, :], msc_sb[0:1, 0:B], msc_sb[0:1, B:B + OL],
                       start=True, stop=False)
                    for j in range(NC):
                        inst = mm(ps_out[:, :], gp_buf[:, j * B:(j + 1) * B],
                                  ow_sb[:, j * OL:(j + 1) * OL],
                                  start=False, stop=(j == NC - 1))
                    inst.then_inc(osem)          # = r - 1
                # issue-based round marker: outproj MMs are queued in-order
                # on the PE; any remote overwrite of their gather source is
                # >=2 full exchange round-trips (~10us) away, while the queued
                # MMs drain in <3us — see module docstring
                te.sem_inc(pesem, 1)             # = r
                dummy(NDUM)

            # round S+1: out-projection of h_S
            r = S_ + 1
            gp_buf = gather[(r - 1) % 2]
            for j in range(NC):
                te.wait_ge(gsems[j], 2 * r)
            te.wait_ge(ocp, r - 2)
            mm(ps_out[:, :], msc_sb[0:1, 0:B], msc_sb[0:1, B:B + OL],
               start=True, stop=False)
            for j in range(NC):
                inst = mm(ps_out[:, :], gp_buf[:, j * B:(j + 1) * B],
                          ow_sb[:, j * OL:(j + 1) * OL],
                          start=False, stop=(j == NC - 1))
            inst.then_inc(osem)                  # = S

        @block.scalar
        def _(act):
            act.wait_ge(in_dma, 128)
            act.wait_ge(mmq, 1)
            act.activation(hbf[0][:, :], ps_h0[:, :], AF.Identity,
                           bias=bia_sb[:, 4:5]).then_inc(hsem)   # h_0
            for r in range(1, S_ + 1):
                base = 1 + 4 * (r - 1)
                for t, (buf, fn, bc) in enumerate((
                        (f_sb, AF.Sigmoid, 0), (i_sb, AF.Sigmoid, 1),
                        (g_sb, AF.Tanh, 2), (o_sb, AF.Sigmoid, 3))):
                    act.wait_ge(mmq, base + t + 1)
                    act.activation(buf[:, :], ps_t[t][:, :], fn,
                                   bias=bia_sb[:, bc:bc + 1]).then_inc(actsem)
                act.wait_ge(dvesem, r)
                act.activation(tanhc_sb[:, :], c_sb[:, :], AF.Tanh
                               ).then_inc(actsem)                # = 5r

        @block.vector
        def _(dve):
            tt = dve.tensor_tensor
            for r in range(1, S_ + 1):
                if r >= 2:
                    dve.wait_ge(actsem, 5 * (r - 1) + 1)
                    tt(t2_sb[:, :], f_sb[:, :], c_sb[:, :], ALU.mult)
                dve.wait_ge(actsem, 5 * (r - 1) + 3)
                if r == 1:
                    tt(c_sb[:, :], i_sb[:, :], g_sb[:, :],
                       ALU.mult).then_inc(dvesem)                # = 1
                else:
                    tt(t1_sb[:, :], i_sb[:, :], g_sb[:, :], ALU.mult)
                    tt(c_sb[:, :], t1_sb[:, :], t2_sb[:, :],
                       ALU.add).then_inc(dvesem)                 # = r
                dve.wait_ge(actsem, 5 * r)
                if r >= 2:
                    dve.wait_ge(lsem, 16 * (r - 1))   # hbf[r%2] sent out
                tt(hbf[r % 2][:, :], o_sb[:, :], tanhc_sb[:, :],
                   ALU.mult).then_inc(hsem)                      # = r + 1
                if r >= 2:
                    dve.wait_ge(osem, r - 1)
                    dve.tensor_copy(out_acc[:, (r - 2) * OL:(r - 1) * OL],
                                    ps_out[:, :]).then_inc(ocp)  # = r - 1
            dve.wait_ge(osem, S_)
            dve.tensor_copy(out_acc[:, (S_ - 1) * OL:S_ * OL],
                            ps_out[:, :]).then_inc(ocp)          # = S

    ctx.close()
    nc.finalize()
    return nc


def _prep_inputs(latent, fc_w, fc_b, w_ih, w_hh, b_ih, b_hh, out_w, out_b):
    latent = np.asarray(latent, np.float32)
    fc_w = np.asarray(fc_w, np.float32)
    fc_b = np.asarray(fc_b, np.float32)
    w_ih = np.asarray(w_ih, np.float32)
    w_hh = np.asarray(w_hh, np.float32)
    b_ih = np.asarray(b_ih, np.float32)
    b_hh = np.asarray(b_hh, np.float32)
    out_w = np.asarray(out_w, np.float32)
    out_b = np.asarray(out_b, np.float32)

    w_comb = w_ih + w_hh
    bias = b_ih + b_hh

    latT = np.zeros((128, 2 * B), np.float32)
    for t in range(2):
        latT[:, t * B:(t + 1) * B] = latent[:, t * 128:(t + 1) * 128].T

    # gate row blocks in reference order i,f,g,o; tile order here: f,i,g,o
    tile_block = [1, 0, 2, 3]

    in_maps = []
    for q in range(NC):
        hsl = slice(HL * q, HL * (q + 1))
        wcT = np.zeros((128, 32 * 128), np.float32)
        whhT = np.zeros((128, 32 * 128), np.float32)
        biasv = np.zeros((128, 8), np.float32)
        for t in range(4):
            rows = slice(tile_block[t] * H + HL * q,
                         tile_block[t] * H + HL * (q + 1))
            biasv[:, t] = bias[rows]
            for j in range(NC):
                ksl = slice(128 * j, 128 * (j + 1))
                col = (t * 8 + j) * 128
                wcT[:, col:col + 128] = w_comb[rows, ksl].T
                whhT[:, col:col + 128] = w_hh[rows, ksl].T
        biasv[:, 4] = fc_b[hsl]
        fcwT = np.zeros((128, 2 * HL), np.float32)
        for t in range(2):
            fcwT[:, t * HL:(t + 1) * HL] = fc_w[hsl, t * 128:(t + 1) * 128].T
        owT = np.zeros((128, NC * OL), np.float32)
        for j in range(NC):
            ksl = slice(128 * j, 128 * (j + 1))
            owT[:, j * OL:(j + 1) * OL] = out_w[OL * q:OL * (q + 1), ksl].T
        msc = np.zeros((1, 192), np.float32)
        msc[0, 0:B] = 1.0
        msc[0, B:B + OL] = out_b[OL * q:OL * (q + 1)]
        in_maps.append({
            "latT": latT.astype(BF16),
            "fcwT": fcwT.astype(BF16),
            "biasv": biasv,
            "msc": msc.astype(BF16),
            "rank": np.array([[q, 0]], np.int32),
            "whhT": whhT.astype(BF16),
            "wcT": wcT.astype(BF16),
            "owT": owT.astype(BF16),
        })
    return in_maps


def _install_profile_shim():
    import types
    if 'antenv.axon_hooks' in sys.modules:
        return
    m = types.ModuleType('antenv.axon_hooks')
    m._hook = None
    m.set_axon_ntff_profile_hook = lambda h: setattr(m, '_hook', h)
    m.get_axon_ntff_profile_hook = lambda: m._hook
    sys.modules['antenv.axon_hooks'] = m
    try:
        import antenv
        antenv.axon_hooks = m
        from trn_agent_boot.trn_boot import _ntff_profile_via_ctypes
        m.set_axon_ntff_profile_hook(
            _ntff_profile_via_ctypes('/opt/axon/libaxon_pjrt.so'))
    except Exception:
        pass


_CACHE = {}


def kernel(latent, seq_len, fc_w, fc_b, w_ih, w_hh, b_ih, b_hh, out_w, out_b):
    from concourse import bass_utils

    s_len = int(seq_len)
    assert s_len == S, f"kernel hardcodes seq_len={S}, got {s_len}"

    if os.environ.get("BASS_TRACE"):
        _install_profile_shim()

    if "nc" not in _CACHE:
        _CACHE["nc"] = _build_nc(s_len)
    nc = _CACHE["nc"]

    in_maps = _prep_inputs(latent, fc_w, fc_b, w_ih, w_hh, b_ih, b_hh,
                           out_w, out_b)

    kw = {}
    if os.environ.get("BASS_TRACE"):
        import tempfile
        kw["trace"] = True
        kw["tmpdir"] = tempfile.mkdtemp(prefix="nn_decoder_")
        print(f"[kernel] trace tmpdir: {kw['tmpdir']}")
    res = bass_utils.run_bass_kernel_spmd(
        nc, in_maps, core_ids=list(range(NC)), **kw)
    if getattr(res, "exec_time_ns", None) is not None:
        print(f"[kernel] exec_time_ns: {res.exec_time_ns}")
        _CACHE["exec_time_ns"] = res.exec_time_ns

    outs = [np.asarray(res.results[q]["out"], np.float32).reshape(B, s_len, OL)
            for q in range(NC)]
    return np.concatenate(outs, axis=2)


# revision 6
# speedup vs baseline: 1.0859x; 1.0859x over previous
"""LSTM decoder (nn_Decoder) on 8 trn2 NeuronCores — remote-DMA exchange.

Tensor-parallel over the 4H gate dimension, with all state in transposed
([channel, batch]) layout so the per-step hidden-state all-gather is a
direct SBUF->SBUF remote_dma_broadcast (no ncfw collective, no DRAM
bounce, no transpose DMAs).

Reference recurrence: x_t = h_t after step 1, so steps >= 2 use
W = w_ih + w_hh (host-side sum); step 1 (x0 = 0) uses w_hh alone.

Per core (rank q owns H-channels [128q, 128q+128) and their 4 gate rows):
  - gates.T tiles f,i,g,o: 32 matmuls, stationary = weight tile [128,128]
    bf16 (FWL), moving = gathered h.T chunk [128, 64]; arrival waits are
    interleaved into the first tile so they hide under the matmul stream.
  - gate biases ride the ACT activations as per-partition bias vectors.
  - c/h updates on [128, 64] tiles (DVE), h emitted as bf16 [128, 64].
  - h.T chunk broadcast to slot q of a double-buffered [128, 512] gather
    buffer on all 8 cores (If-chain on a rank register; per-sender remote
    semaphores). Descriptors are prepped during the previous step's
    flight; the trigger is gated on the DVE c-write (the tanh_c + h-mult
    ~0.65us run concurrently with the ~1.3us Q7-observe + trigger + SDMA
    descriptor fetch, so the hbf write still wins by ~0.6us).
  - output projection out[t] = h_{t+1} @ out_w_q.T rides the gather slots
    as stationary one round behind; out_b via a K=1 ones-matmul. Dummy
    matmuls keep the PE HAM clock warm through the exchange window.
  - the ncfw AllGather at the top is never waited on mid-path: its mere
    presence makes the runtime align core starts with a pre-exec barrier.

Measured: ~1.44 ms (vs 4.78 ms ncfw-collective baseline), rel err 4.5e-3.
"""
import os
import sys

sys.path.insert(0, "/opt/trn_rl_repo")

import numpy as np
import ml_dtypes

BF16 = ml_dtypes.bfloat16

B = 64          # batch
L = 256         # latent dim
H = 1024        # hidden
O = 512         # output dim
S = 256         # seq len
NC = 8          # cores
HL = H // NC    # 128, per-core h slice
OL = O // NC    # 64, per-core out slice
NDUM = 12        # dummy matmuls per round (HAM warmth during flight)


def _build_nc(s_len):
    from concourse import bass, mybir
    from concourse import bacc

    S_ = s_len
    nc = bacc.Bacc("TRN2", debug=False)
    f32 = mybir.dt.float32
    bf16 = mybir.dt.bfloat16
    i32 = mybir.dt.int32
    AF = mybir.ActivationFunctionType
    ALU = mybir.AluOpType

    d_lat = nc.dram_tensor("latT", [128, 2 * B], bf16, kind="ExternalInput")
    d_fcw = nc.dram_tensor("fcwT", [128, 2 * HL], bf16, kind="ExternalInput")
    d_bia = nc.dram_tensor("biasv", [128, 8], f32, kind="ExternalInput")
    d_msc = nc.dram_tensor("msc", [1, 192], bf16, kind="ExternalInput")
    d_rank = nc.dram_tensor("rank", [1, 2], i32, kind="ExternalInput")
    d_whh = nc.dram_tensor("whhT", [128, 32 * 128], bf16, kind="ExternalInput")
    d_wc = nc.dram_tensor("wcT", [128, 32 * 128], bf16, kind="ExternalInput")
    d_ow = nc.dram_tensor("owT", [128, NC * OL], bf16, kind="ExternalInput")
    d_out = nc.dram_tensor("out", [B, S_ * OL], f32, kind="ExternalOutput")

    cc_in = nc.dram_tensor("cc_in", [1, 8], f32)
    cc_out = nc.dram_tensor("cc_out", [8, 8], f32, addr_space="Shared")

    from contextlib import ExitStack
    ctx = ExitStack()
    sem = lambda n: ctx.enter_context(nc.semaphore(n))
    sb = lambda n, sh, dt: ctx.enter_context(nc.sbuf_tensor(n, sh, dt))
    ps = lambda n, sh, dt: ctx.enter_context(nc.psum_tensor(n, sh, dt))

    in_dma = sem("in_dma")    # input loads, +16 each
    bsem = sem("bsem")        # startup barrier
    gsems = [sem(f"gsem{q}") for q in range(NC)]  # +2 per exchange from rank q
    psem = sem("psem")        # desc-gen done, = e+1
    lsem = sem("lsem")        # send complete, = 16(e+1)
    hsem = sem("hsem")        # h_e ready, = e+1
    mmq = sem("mmq")          # PE: h0 -> 1; round r tile t -> 1+4(r-1)+t+1
    actsem = sem("actsem")    # ACT: 5 per round
    dvesem = sem("dvesem")    # DVE c ready, = r
    osem = sem("osem")        # outproj psum done, = r-1
    ocp = sem("ocp")          # out copy done, = r-1
    pesem = sem("pesem")      # PE round done (incl outproj), = r
    odma = sem("odma")        # output flush dmas

    lat_sb = sb("lat_sb", [128, 2 * B], bf16)
    fcw_sb = sb("fcw_sb", [128, 2 * HL], bf16)
    bia_sb = sb("bia_sb", [128, 8], f32)
    msc_sb = sb("msc_sb", [1, 192], bf16)
    rank_sb = sb("rank_sb", [1, 2], i32)
    whh_sb = sb("whh_sb", [128, 32 * 128], bf16)
    wc_sb = sb("wc_sb", [128, 32 * 128], bf16)
    ow_sb = sb("ow_sb", [128, NC * OL], bf16)
    gather = [sb("gather0", [128, NC * B], bf16),
              sb("gather1", [128, NC * B], bf16)]
    hbf = [sb("hbf0", [128, B], bf16), sb("hbf1", [128, B], bf16)]
    f_sb = sb("f_sb", [128, B], f32)
    i_sb = sb("i_sb", [128, B], f32)
    g_sb = sb("g_sb", [128, B], f32)
    o_sb = sb("o_sb", [128, B], f32)
    t1_sb = sb("t1_sb", [128, B], f32)
    t2_sb = sb("t2_sb", [128, B], f32)
    c_sb = sb("c_sb", [128, B], f32)
    tanhc_sb = sb("tanhc_sb", [128, B], f32)
    out_acc = sb("out_acc", [B, S_ * OL], f32)

    ps_t = [ps(f"ps_{n}", [128, B], f32) for n in "figo"]
    ps_h0 = ps("ps_h0", [128, B], f32)
    ps_out = ps("ps_out", [B, OL], f32)
    ps_dum = ps("ps_dum", [128, 512], f32)

    OUT_CHUNK = 32
    n_chunks = S_ // OUT_CHUNK

    with nc.Block() as block:

        @block.sync
        def _(sync):
            for dram, buf in ((d_lat, lat_sb), (d_fcw, fcw_sb),
                              (d_bia, bia_sb), (d_msc, msc_sb),
                              (d_rank, rank_sb), (d_whh, whh_sb),
                              (d_wc, wc_sb), (d_ow, ow_sb)):
                sync.dma_start(buf[:, :], dram[:, :]).then_inc(in_dma, 16)
            # two-part flush: first half mid-loop (one contention event),
            # second half at the end (shorter tail)
            HF = S_ // 2
            sync.wait_ge(ocp, HF)
            sync.dma_start(d_out[:, :HF * OL],
                           out_acc[:, :HF * OL]).then_inc(odma, 16)
            sync.wait_ge(ocp, S_)
            sync.dma_start(d_out[:, HF * OL:],
                           out_acc[:, HF * OL:]).then_inc(odma, 16)
            sync.wait_ge(odma, 32)

        @block.gpsimd
        def _(gp):
            gp.collective_compute(
                "AllGather", mybir.AluOpType.bypass,
                replica_groups=[list(range(NC))],
                ins=[cc_in.ap().opt()], outs=[cc_out.ap().opt()],
            ).then_inc(bsem)
            # no wait on bsem here: the NEFF-level pre-execution barrier
            # (present because this collective exists) already aligns core
            # starts; exchanges ride plain SDMA and need no ncfw
            gp.wait_ge(in_dma, 128)          # all inputs loaded
            r = gp.alloc_register("rankreg")
            gp.reg_load(r, rank_sb[0:1, 0:1])
            rdests = [(0, k) for k in range(NC)]
            for e in range(S_ + 1):
                gb = gather[e % 2]

                def bc(q):
                    gp.remote_dma_broadcast(
                        gb[:, q * B:(q + 1) * B], hbf[e % 2][:, :],
                        remote_sem=gsems[q], local_sem=lsem, rdests=rdests,
                    ).then_inc(psem, 1)

                def chain(lo, hi):
                    if hi - lo == 1:
                        bc(lo)
                        return
                    mid = (lo + hi) // 2
                    with gp.If_lt(r, mid):
                        chain(lo, mid)
                    with gp.Else():
                        chain(mid, hi)

                chain(0, NC)
                gp.wait_ge(psem, e + 1)
                if e >= 2:
                    for q in range(NC):
                        gp.wait_ge(gsems[q], 2 * (e - 1))
                if e >= 2:
                    gp.wait_ge(lsem, 16 * (e - 1))
                if e >= 1:
                    gp.wait_ge(pesem, e)     # peers' gather readers done
                if e == 0:
                    gp.wait_ge(hsem, 1)      # h_0 ready
                else:
                    # early trigger: c done; tanh_c + h-mult (~0.65us) run
                    # while the Q7 observe+trigger+SDMA descriptor fetch
                    # (~1.3us) proceeds — hbf write wins by ~0.6us
                    gp.wait_ge(dvesem, e)
                gp.trigger_dma(count=1)
            gp.wait_ge(bsem, 1)

        @block.tensor
        def _(te):
            mm = te.matmul
            te.wait_ge(in_dma, 128)
            mm(ps_h0[:, :], fcw_sb[:, 0:HL], lat_sb[:, 0:B],
               start=True, stop=False)
            mm(ps_h0[:, :], fcw_sb[:, HL:2 * HL], lat_sb[:, B:2 * B],
               start=False, stop=True).then_inc(mmq)       # mmq = 1

            def dummy(n):
                for _ in range(n):
                    mm(ps_dum[:, :], wc_sb[:, 0:128], wc_sb[:, 128:640],
                       start=True, stop=True)

            def small_dummy():
                return mm(ps_dum[:, 0:B], wc_sb[:, 0:128], wc_sb[:, 128:192],
                          start=True, stop=True)

            for r in range(1, S_ + 1):
                gp_buf = gather[(r - 1) % 2]
                W = whh_sb if r == 1 else wc_sb
                for t in range(4):
                    for j in range(NC):
                        if t == 0:
                            # interleaved arrival waits: wait j hides under MM j-1
                            te.wait_ge(gsems[j], 2 * r)
                        inst = mm(ps_t[t][:, :],
                                  W[:, (t * 8 + j) * 128:(t * 8 + j + 1) * 128],
                                  gp_buf[:, j * B:(j + 1) * B],
                                  start=(j == 0), stop=(j == NC - 1))
                        if j == NC - 1:
                            inst.then_inc(mmq)   # = 1 + 4(r-1) + t + 1
                if r >= 2:
                    te.wait_ge(ocp, r - 2)
                    mm(ps_out[:, :], msc_sb[0:1, 0:B], msc_sb[0:1, B:B + OL],
                       start=True, stop=False)
                    for j in range(NC):
                        inst = mm(ps_out[:, :], gp_buf[:, j * B:(j + 1) * B],
                                  ow_sb[:, j * OL:(j + 1) * OL],
                                  start=False, stop=(j == NC - 1))
                    inst.then_inc(osem)          # = r - 1
                # issue-based round marker: outproj MMs are queued in-order
                # on the PE; any remote overwrite of their gather source is
                # >=2 full exchange round-trips (~10us) away, while the queued
                # MMs drain in <3us — see module docstring
                te.sem_inc(pesem, 1)             # = r
                dummy(NDUM)

            # round S+1: out-projection of h_S
            r = S_ + 1
            gp_buf = gather[(r - 1) % 2]
            for j in range(NC):
                te.wait_ge(gsems[j], 2 * r)
            te.wait_ge(ocp, r - 2)
            mm(ps_out[:, :], msc_sb[0:1, 0:B], msc_sb[0:1, B:B + OL],
               start=True, stop=False)
            for j in range(NC):
                inst = mm(ps_out[:, :], gp_buf[:, j * B:(j + 1) * B],
                          ow_sb[:, j * OL:(j + 1) * OL],
                          start=False, stop=(j == NC - 1))
            inst.then_inc(osem)                  # = S

        @block.scalar
        def _(act):
            act.wait_ge(in_dma, 128)
            act.wait_ge(mmq, 1)
            act.activation(hbf[0][:, :], ps_h0[:, :], AF.Identity,
                           bias=bia_sb[:, 4:5]).then_inc(hsem)   # h_0
            for r in range(1, S_ + 1):
                base = 1 + 4 * (r - 1)
                for t, (buf, fn, bc) in enumerate((
                        (f_sb, AF.Sigmoid, 0), (i_sb, AF.Sigmoid, 1),
                        (g_sb, AF.Tanh, 2), (o_sb, AF.Sigmoid, 3))):
                    act.wait_ge(mmq, base + t + 1)
                    act.activation(buf[:, :], ps_t[t][:, :], fn,
                                   bias=bia_sb[:, bc:bc + 1]).then_inc(actsem)
                act.wait_ge(dvesem, r)
                act.activation(tanhc_sb[:, :], c_sb[:, :], AF.Tanh
                               ).then_inc(actsem)                # = 5r

        @block.vector
        def _(dve):
            tt = dve.tensor_tensor
            for r in range(1, S_ + 1):
                if r >= 2:
                    dve.wait_ge(actsem, 5 * (r - 1) + 1)
                    tt(t2_sb[:, :], f_sb[:, :], c_sb[:, :], ALU.mult)
                dve.wait_ge(actsem, 5 * (r - 1) + 3)
                if r == 1:
                    tt(c_sb[:, :], i_sb[:, :], g_sb[:, :],
                       ALU.mult).then_inc(dvesem)                # = 1
                else:
                    tt(t1_sb[:, :], i_sb[:, :], g_sb[:, :], ALU.mult)
                    tt(c_sb[:, :], t1_sb[:, :], t2_sb[:, :],
                       ALU.add).then_inc(dvesem)                 # = r
                dve.wait_ge(actsem, 5 * r)
                if r >= 2:
                    dve.wait_ge(lsem, 16 * (r - 1))   # hbf[r%2] sent out
                tt(hbf[r % 2][:, :], o_sb[:, :], tanhc_sb[:, :],
                   ALU.mult).then_inc(hsem)                      # = r + 1
                if r >= 2:
                    dve.wait_ge(osem, r - 1)
                    dve.tensor_copy(out_acc[:, (r - 2) * OL:(r - 1) * OL],
                                    ps_out[:, :]).then_inc(ocp)  # = r - 1
            dve.wait_ge(osem, S_)
            dve.tensor_copy(out_acc[:, (S_ - 1) * OL:S_ * OL],
                            ps_out[:, :]).then_inc(ocp)          # = S

    ctx.close()
    nc.finalize()
    return nc


def _prep_inputs(latent, fc_w, fc_b, w_ih, w_hh, b_ih, b_hh, out_w, out_b):
    latent = np.asarray(latent, np.float32)
    fc_w = np.asarray(fc_w, np.float32)
    fc_b = np.asarray(fc_b, np.float32)
    w_ih = np.asarray(w_ih, np.float32)
    w_hh = np.asarray(w_hh, np.float32)
    b_ih = np.asarray(b_ih, np.float32)
    b_hh = np.asarray(b_hh, np.float32)
    out_w = np.asarray(out_w, np.float32)
    out_b = np.asarray(out_b, np.float32)

    w_comb = w_ih + w_hh
    bias = b_ih + b_hh

    latT = np.zeros((128, 2 * B), np.float32)
    for t in range(2):
        latT[:, t * B:(t + 1) * B] = latent[:, t * 128:(t + 1) * 128].T

    # gate row blocks in reference order i,f,g,o; tile order here: f,i,g,o
    tile_block = [1, 0, 2, 3]

    in_maps = []
    for q in range(NC):
        hsl = slice(HL * q, HL * (q + 1))
        wcT = np.zeros((128, 32 * 128), np.float32)
        whhT = np.zeros((128, 32 * 128), np.float32)
        biasv = np.zeros((128, 8), np.float32)
        for t in range(4):
            rows = slice(tile_block[t] * H + HL * q,
                         tile_block[t] * H + HL * (q + 1))
            biasv[:, t] = bias[rows]
            for j in range(NC):
                ksl = slice(128 * j, 128 * (j + 1))
                col = (t * 8 + j) * 128
                wcT[:, col:col + 128] = w_comb[rows, ksl].T
                whhT[:, col:col + 128] = w_hh[rows, ksl].T
        biasv[:, 4] = fc_b[hsl]
        fcwT = np.zeros((128, 2 * HL), np.float32)
        for t in range(2):
            fcwT[:, t * HL:(t + 1) * HL] = fc_w[hsl, t * 128:(t + 1) * 128].T
        owT = np.zeros((128, NC * OL), np.float32)
        for j in range(NC):
            ksl = slice(128 * j, 128 * (j + 1))
            owT[:, j * OL:(j + 1) * OL] = out_w[OL * q:OL * (q + 1), ksl].T
        msc = np.zeros((1, 192), np.float32)
        msc[0, 0:B] = 1.0
        msc[0, B:B + OL] = out_b[OL * q:OL * (q + 1)]
        in_maps.append({
            "latT": latT.astype(BF16),
            "fcwT": fcwT.astype(BF16),
            "biasv": biasv,
            "msc": msc.astype(BF16),
            "rank": np.array([[q, 0]], np.int32),
            "whhT": whhT.astype(BF16),
            "wcT": wcT.astype(BF16),
            "owT": owT.astype(BF16),
        })
    return in_maps


def _install_profile_shim():
    import types
    if 'antenv.axon_hooks' in sys.modules:
        return
    m = types.ModuleType('antenv.axon_hooks')
    m._hook = None
    m.set_axon_ntff_profile_hook = lambda h: setattr(m, '_hook', h)
    m.get_axon_ntff_profile_hook = lambda: m._hook
    sys.modules['antenv.axon_hooks'] = m
    try:
        import antenv
        antenv.axon_hooks = m
        from trn_agent_boot.trn_boot import _ntff_profile_via_ctypes
        m.set_axon_ntff_profile_hook(
            _ntff_profile_via_ctypes('/opt/axon/libaxon_pjrt.so'))
    except Exception:
        pass


_CACHE = {}


def kernel(latent, seq_len, fc_w, fc_b, w_ih, w_hh, b_ih, b_hh, out_w, out_b):
    from concourse import bass_utils

    s_len = int(seq_len)
    assert s_len == S, f"kernel hardcodes seq_len={S}, got {s_len}"

    if os.environ.get("BASS_TRACE"):
        _install_profile_shim()

    if "nc" not in _CACHE:
        _CACHE["nc"] = _build_nc(s_len)
    nc = _CACHE["nc"]

    in_maps = _prep_inputs(latent, fc_w, fc_b, w_ih, w_hh, b_ih, b_hh,
                           out_w, out_b)

    kw = {}
    if os.environ.get("BASS_TRACE"):
        import tempfile
        kw["trace"] = True
        kw["tmpdir"] = tempfile.mkdtemp(prefix="nn_decoder_")
        print(f"[kernel] trace tmpdir: {kw['tmpdir']}")
    res = bass_utils.run_bass_kernel_spmd(
        nc, in_maps, core_ids=list(range(NC)), **kw)
    if getattr(res, "exec_time_ns", None) is not None:
        print(f"[kernel] exec_time_ns: {res.exec_time_ns}")
        _CACHE["exec_time_ns"] = res.exec_time_ns

    outs = [np.asarray(res.results[q]["out"], np.float32).reshape(B, s_len, OL)
            for q in range(NC)]
    return np.concatenate(outs, axis=2)
